# revision 3
# baseline (speedup 1.0000x reference)
"""Trainium2 Bass kernel for nn_BatchRankingLoss (n=8192, 8 NeuronCores).

Math: reference computes sum over pairs i<j of relu(-(p_j-p_i)*sign(l_j-l_i) + 2).
The sum runs over UNORDERED pairs and is invariant to re-indexing, so we sort by
labels on the host: with q = preds[argsort(labels)], the loss becomes
    sum_{u<v} relu(2 + q_u - q_v)
(plus an exact O(#ties) host correction for tied labels, where sign()=0).

Device strategy (SPMD, 8 cores, one shared program):
- 64 row-tiles of 128 rows. Core k gets tiles {k+16m, 15-k+16m}, presented to the
  program as 8 fixed-width "slots" of [16,14,12,10,8,6,4,2] 512-col chunks.
  Each slot's column window starts at its diagonal block; unused tail columns are
  zero-padded in the rhs data (rhs rows both 0 -> t=0 -> contributes 0 exactly).
- PE: one K=2 float32r matmul per 512-col bank produces t = q_u + (2 - q_v) in
  PSUM; the diagonal bank gets a second accumulating matmul adding -1e9 on the
  lower triangle (tri/pen constant operands), masking j<=i pairs.
- ACT lane: activation(Relu, accum_out) -> per-partition sum of relu(t).
- DVE lane: tensor_reduce(add, |.|) -> per-partition sum of |t|; combined with the
  analytic linear term sum(t) (per-slot affine in q_u, passed as per-core inputs)
  via relu(t) = (t + |t|)/2.
- Each core outputs a [128,1] partial; host sums 8x128 partials + tie correction.
"""

import numpy as np

N = 8192
NBLK = 64                      # 128-row tiles
SLOT_CHUNKS = [16, 14, 12, 10, 8, 6, 4, 2]   # 512-col chunks per slot
NCOLS = sum(c * 512 for c in SLOT_CHUNKS)     # 36864 padded cols per core
GROUP_CHUNKS = 2                               # chunks per reduce group (2 banks)
PENALTY = -1.0e9

# ---------------------------------------------------------------------------
# Build the group schedule (shared by program builder and host linear terms)
# ---------------------------------------------------------------------------

def make_schedule():
    """Returns list of groups: (slot, col_off_in_slot, width, is_diag, engine).

    engine: 'A' (ACT relu path) or 'D' (DVE abs path). Diagonal groups are
    forced to ACT (penalty makes masked t ~ -1e9; relu kills it exactly, abs
    would not). Remaining groups greedily balance estimated engine time.
    """
    groups = []
    for s, nchunks in enumerate(SLOT_CHUNKS):
        off = 0
        first = True
        left = nchunks
        while left > 0:
            g = min(GROUP_CHUNKS, left)
            groups.append([s, off, g * 512, first])
            off += g * 512
            left -= g
            first = False

    act_cost = 0.0
    dve_cost = 0.0
    sched = []
    for s, off, w, diag in groups:
        ca = w * 0.8333 + 290.0
        cd = w * 1.0417 + 170.0
        if diag:
            eng = "A"
        else:
            eng = "A" if act_cost + ca <= dve_cost + cd else "D"
        if eng == "A":
            act_cost += ca
        else:
            dve_cost += cd
        sched.append((s, off, w, diag, eng))
    return sched

SCHEDULE = make_schedule()
SLOT_OFFS = []
_o = 0
for _c in SLOT_CHUNKS:
    SLOT_OFFS.append(_o)
    _o += _c * 512
del _o, _c

def slot_col_offsets():
    offs = []
    o = 0
    for c in SLOT_CHUNKS:
        offs.append(o)
        o += c * 512
    return offs

# ---------------------------------------------------------------------------
# Device program
# ---------------------------------------------------------------------------

_CACHE = {}

def build_program():
    import concourse.bacc as bacc
    import concourse.mybir as mybir
    from concourse.tile import TileContext

    F32 = mybir.dt.float32
    F32R = mybir.dt.float32r
    AX = mybir.AxisListType
    OP = mybir.AluOpType
    AF = mybir.ActivationFunctionType

    nA = sum(1 for g in SCHEDULE if g[4] == "A")
    nD = sum(1 for g in SCHEDULE if g[4] == "D")

    nc = bacc.Bacc(trn_type="TRN2")
    rhs_d = nc.dram_tensor("rhs", [2, NCOLS], F32R, kind="ExternalInput")
    lhs_d = nc.dram_tensor("lhs", [2, 1024], F32R, kind="ExternalInput")
    tri_d = nc.dram_tensor("tri", [128, 128], F32R, kind="ExternalInput")
    pen_d = nc.dram_tensor("pen", [128, 512], F32R, kind="ExternalInput")
    qcol_d = nc.dram_tensor("qcol", [128, 8], F32, kind="ExternalInput")
    lin_d = nc.dram_tensor("linab", [128, 16], F32, kind="ExternalInput")
    out_d = nc.dram_tensor("out", [128, 1], F32, kind="ExternalOutput")

    with TileContext(nc) as tc:
        with tc.tile_pool(name="consts", bufs=1) as cpool, \
             tc.tile_pool(name="scr", bufs=2) as spool, \
             tc.tile_pool(name="ps", bufs=4, space="PSUM") as psp:
            RHS = cpool.tile([2, NCOLS], F32R)
            LHS = cpool.tile([2, 1024], F32R)
            TRI = cpool.tile([128, 128], F32R)
            PEN = cpool.tile([128, 512], F32R)
            QCOL = cpool.tile([128, 8], F32)
            LIN = cpool.tile([128, 16], F32)
            ACCA = cpool.tile([128, nA], F32)
            ACCD = cpool.tile([128, nD], F32)
            ACCL = cpool.tile([128, 8], F32)
            R = cpool.tile([128, 4], F32)

            # slot-0 window first so its compute overlaps the rest of the load
            nc.sync.dma_start(out=RHS[:, :8192], in_=rhs_d[:, :8192])
            nc.sync.dma_start(out=RHS[:, 8192:], in_=rhs_d[:, 8192:])
            nc.sync.dma_start(out=LHS[:], in_=lhs_d[:])
            nc.sync.dma_start(out=TRI[:], in_=tri_d[:])
            nc.sync.dma_start(out=PEN[:], in_=pen_d[:])
            nc.sync.dma_start(out=QCOL[:], in_=qcol_d[:])
            nc.sync.dma_start(out=LIN[:], in_=lin_d[:])

            ia = 0
            id_ = 0
            for (s, off, w, diag, eng) in SCHEDULE:
                PS = psp.tile([128, GROUP_CHUNKS * 512], F32, tag="ps")
                lhsT = LHS[:, s * 128:(s + 1) * 128]
                for b in range(w // 512):
                    c0 = SLOT_OFFS[s] + off + b * 512
                    nc.tensor.matmul(PS[:, b * 512:(b + 1) * 512], lhsT,
                                     RHS[:, c0:c0 + 512],
                                     start=True, stop=not (diag and b == 0))
                if diag:
                    nc.tensor.matmul(PS[:, 0:512], TRI[:], PEN[:],
                                     start=False, stop=True)
                if eng == "A":
                    SCR = spool.tile([128, GROUP_CHUNKS * 512], F32, tag="scr")
                    nc.scalar.activation(out=SCR[:, :w], in_=PS[:, :w], func=AF.Relu,
                                         bias=0.0, scale=1.0,
                                         accum_out=ACCA[:, ia:ia + 1])
                    ia += 1
                else:
                    nc.vector.tensor_reduce(out=ACCD[:, id_:id_ + 1], in_=PS[:, :w],
                                            axis=AX.X, op=OP.add,
                                            apply_absolute_value=True)
                    id_ += 1

            # linear terms: accL[:, s] = A_s * q_u + B_s   (A,B per-core inputs)
            for s in range(8):
                nc.vector.tensor_scalar(ACCL[:, s:s + 1], QCOL[:, s:s + 1],
                                        LIN[:, 2 * s:2 * s + 1],
                                        LIN[:, 2 * s + 1:2 * s + 2],
                                        OP.mult, OP.add)

            # combine: out = sum(ACCA) + 0.5*(sum(ACCD) + sum(ACCL))
            nc.vector.tensor_reduce(out=R[:, 0:1], in_=ACCA[:], axis=AX.X, op=OP.add)
            nc.vector.tensor_reduce(out=R[:, 1:2], in_=ACCD[:], axis=AX.X, op=OP.add)
            nc.vector.tensor_reduce(out=R[:, 2:3], in_=ACCL[:], axis=AX.X, op=OP.add)
            nc.vector.tensor_tensor(out=R[:, 1:2], in0=R[:, 1:2], in1=R[:, 2:3],
                                    op=OP.add)
            nc.vector.tensor_scalar(R[:, 1:2], R[:, 1:2], 0.5, None, OP.mult)
            nc.vector.tensor_tensor(out=R[:, 0:1], in0=R[:, 0:1], in1=R[:, 1:2],
                                    op=OP.add)
            OUT = cpool.tile([128, 1], F32)
            nc.vector.tensor_copy(out=OUT[:], in_=R[:, 0:1])
            nc.sync.dma_start(out=out_d[:], in_=OUT[:])

    nc.finalize()
    return nc


def get_program():
    if "nc" not in _CACHE:
        _CACHE["nc"] = build_program()
    return _CACHE["nc"]

# ---------------------------------------------------------------------------
# Host side
# ---------------------------------------------------------------------------

def core_tiles(k):
    """Row-tiles for core k, widest first (matches slot widths)."""
    ts = sorted([k + 16 * m for m in range(4)] + [15 - k + 16 * m for m in range(4)])
    return ts


def build_inputs(q):
    """Per-core in_maps for label-sorted preds q (np.float32 [8192])."""
    q = q.astype(np.float32)
    rhs_row1_full = (2.0 - q).astype(np.float32)     # global 2 - q_v
    tri = np.triu(np.ones((128, 128), np.float32))    # tri[k,i] = 1 if k<=i
    pen = np.zeros((128, 512), np.float32)
    pen[np.arange(128), np.arange(128)] = PENALTY

    offs = slot_col_offsets()
    in_maps = []
    for k in range(8):
        tiles = core_tiles(k)
        rhs = np.zeros((2, NCOLS), np.float32)
        lhs = np.zeros((2, 1024), np.float32)
        qcol = np.zeros((128, 8), np.float32)
        lin = np.zeros((128, 16), np.float32)
        for s, t in enumerate(tiles):
            real = (NBLK - t) * 128                 # real window width
            wslot = SLOT_CHUNKS[s] * 512
            take = min(real, wslot)
            so = offs[s]
            rhs[0, so:so + take] = 1.0
            rhs[1, so:so + take] = rhs_row1_full[t * 128: t * 128 + take]
            lhs[0, s * 128:(s + 1) * 128] = q[t * 128:(t + 1) * 128]
            lhs[1, s * 128:(s + 1) * 128] = 1.0
            qcol[:, s] = q[t * 128:(t + 1) * 128]
            # linear terms over this slot's DVE groups (f64 accumulate)
            A = 0.0
            B = 0.0
            for (gs, off, w, diag, eng) in SCHEDULE:
                if gs != s or eng != "D":
                    continue
                seg0 = rhs[0, offs[s] + off: offs[s] + off + w].astype(np.float64)
                seg1 = rhs[1, offs[s] + off: offs[s] + off + w].astype(np.float64)
                A += seg0.sum()
                B += seg1.sum()
            lin[:, 2 * s] = np.float32(A)
            lin[:, 2 * s + 1] = np.float32(B)
        in_maps.append({"rhs": rhs, "lhs": lhs, "tri": tri, "pen": pen,
                        "qcol": qcol, "linab": lin})
    return in_maps


def tie_correction(labels, q, order):
    """Exact correction for tied labels: reference uses sign()=0 there."""
    ls = labels[order]
    corr = 0.0
    i = 0
    n = len(ls)
    while i < n:
        j = i + 1
        while j < n and ls[j] == ls[i]:
            j += 1
        if j - i > 1:
            for u in range(i, j):
                for v in range(u + 1, j):
                    corr += 2.0 - max(0.0, 2.0 + float(q[u]) - float(q[v]))
        i = j
    return corr


def run(inputs, trace=False):
    from concourse.bass_utils import run_bass_kernel_spmd

    preds = np.asarray(inputs["preds"], dtype=np.float32)
    labels = np.asarray(inputs["labels"], dtype=np.float32)
    order = np.argsort(labels, kind="stable")
    q = preds[order]

    nc = get_program()
    in_maps = build_inputs(q)
    res = run_bass_kernel_spmd(nc, in_maps, core_ids=list(range(8)), trace=trace)
    total = 0.0
    for c in range(8):
        total += res.results[c]["out"].astype(np.float64).sum()
    total += tie_correction(labels, q, order)
    return np.float32(total), res


def kernel(**inputs):
    out, _ = run(inputs, trace=False)
    return out


# revision 4
# speedup vs baseline: 1.1947x; 1.1947x over previous
"""Trainium2 Bass kernel for nn_BatchRankingLoss (n=8192, 8 NeuronCores).

Math: reference computes sum over pairs i<j of relu(-(p_j-p_i)*sign(l_j-l_i) + 2).
The sum runs over UNORDERED pairs and is invariant to re-indexing, so we sort by
labels on the host: with q = preds[argsort(labels)], the loss becomes
    sum_{u<v} relu(2 + q_u - q_v)
(plus an exact O(#ties) host correction for tied labels, where sign()=0).

Device strategy (SPMD, 8 cores, one shared program):
- 64 row-tiles of 128 rows. Core k gets tiles {k+16m, 15-k+16m}, presented to the
  program as 8 fixed-width "slots" of [16,14,12,10,8,6,4,2] 512-col chunks.
  Each slot's column window starts at its diagonal block; unused tail columns are
  zero in the rhs data (both rhs rows 0 -> t=0 -> contributes 0 exactly).
- rhs data is packed into 8 "streams" on 16 SBUF partitions ([16, 4608] bf16) so
  the load DMA covers 16 partitions; K=16 matmuls with zero-padded lhsT lanes
  select the stream (pair of partitions) each chunk lives on.
- PE: one K=16 bf16 matmul per 512-col chunk produces t = q_u + (2 - q_v) in
  f32 PSUM; the diagonal chunk gets a second accumulating matmul adding -1e9 on
  the lower triangle (tri/pen constant operands), masking j<=i pairs.
- ACT lane: activation(Relu, accum_out) -> per-partition sum of relu(t).
- DVE lane: tensor_reduce(add, |.|) -> per-partition sum of |t|; combined with
  the analytic linear term sum(t) (per-slot affine in q_u, per-core inputs) via
  relu(t) = (t + |t|)/2.
- Each core outputs a [128,1] partial; host sums 8x128 partials + tie correction.
"""

import numpy as np

N = 8192
NBLK = 64                                     # 128-row tiles
SLOT_CHUNKS = [16, 14, 12, 10, 8, 6, 4, 2]    # 512-col chunks per slot
NCHUNKS = sum(SLOT_CHUNKS)                    # 72
STREAM_CAP = 9                                # chunks per stream (9*512 = 4608)
GROUP_CHUNKS = 2                              # chunks per reduce group (2 banks)
PENALTY = -1.0e9

# ---------------------------------------------------------------------------
# Stream packing: (slot, chunk) -> (stream, pos); lhsT variant per (slot,stream)
# ---------------------------------------------------------------------------

def _pack_streams():
    chunk_map = {}           # (slot, chunk_idx) -> (stream, pos)
    variants = []            # list of (slot, stream); index = lhsT variant id
    vmap = {}
    stream = 0
    pos = 0
    for s, nch in enumerate(SLOT_CHUNKS):
        for c in range(nch):
            if pos == STREAM_CAP:
                stream += 1
                pos = 0
            chunk_map[(s, c)] = (stream, pos)
            if (s, stream) not in vmap:
                vmap[(s, stream)] = len(variants)
                variants.append((s, stream))
            pos += 1
    assert stream == 7 and pos == STREAM_CAP
    return chunk_map, variants, vmap

CHUNK_MAP, VARIANTS, VMAP = _pack_streams()
NVAR = len(VARIANTS)

# ---------------------------------------------------------------------------
# Group schedule
# ---------------------------------------------------------------------------

def make_schedule():
    """Groups: (slot, chunk0, nchunks, is_diag, engine 'A'|'D')."""
    groups = []
    for s, nch in enumerate(SLOT_CHUNKS):
        c = 0
        while c < nch:
            g = min(GROUP_CHUNKS, nch - c)
            groups.append([s, c, g, c == 0])
            c += g
    act_cost = 0.0
    dve_cost = 0.0
    sched = []
    for s, c0, g, diag in groups:
        w = g * 512
        ca = w * 0.8333 + 290.0
        cd = w * 1.0417 + 170.0
        if diag:
            eng = "A"
        else:
            eng = "A" if act_cost + ca <= dve_cost + cd else "D"
        if eng == "A":
            act_cost += ca
        else:
            dve_cost += cd
        sched.append((s, c0, g, diag, eng))
    return sched

SCHEDULE = make_schedule()

# ---------------------------------------------------------------------------
# Device program
# ---------------------------------------------------------------------------

_CACHE = {}

def build_program():
    import concourse.bacc as bacc
    import concourse.mybir as mybir
    from concourse.tile import TileContext

    F32 = mybir.dt.float32
    BF16 = mybir.dt.bfloat16
    AX = mybir.AxisListType
    OP = mybir.AluOpType
    AF = mybir.ActivationFunctionType

    nA = sum(1 for g in SCHEDULE if g[4] == "A")
    nD = sum(1 for g in SCHEDULE if g[4] == "D")

    nc = bacc.Bacc(trn_type="TRN2")
    rhs_d = nc.dram_tensor("rhs", [16, STREAM_CAP * 512], BF16, kind="ExternalInput")
    lhs_d = nc.dram_tensor("lhs", [16, NVAR * 128], BF16, kind="ExternalInput")
    tri_d = nc.dram_tensor("tri", [128, 128], BF16, kind="ExternalInput")
    pen_d = nc.dram_tensor("pen", [128, 512], BF16, kind="ExternalInput")
    qcol_d = nc.dram_tensor("qcol", [128, 8], F32, kind="ExternalInput")
    lin_d = nc.dram_tensor("linab", [128, 16], F32, kind="ExternalInput")
    out_d = nc.dram_tensor("out", [128, 1], F32, kind="ExternalOutput")

    with TileContext(nc) as tc:
        with tc.tile_pool(name="consts", bufs=1) as cpool, \
             tc.tile_pool(name="scr", bufs=2) as spool, \
             tc.tile_pool(name="ps", bufs=4, space="PSUM") as psp:
            RHS = cpool.tile([16, STREAM_CAP * 512], BF16)
            LHS = cpool.tile([16, NVAR * 128], BF16)
            TRI = cpool.tile([128, 128], BF16)
            PEN = cpool.tile([128, 512], BF16)
            QCOL = cpool.tile([128, 8], F32)
            LIN = cpool.tile([128, 16], F32)
            ACCA = cpool.tile([128, nA], F32)
            ACCD = cpool.tile([128, nD], F32)
            ACCL = cpool.tile([128, 8], F32)
            R = cpool.tile([128, 4], F32)
            OUT = cpool.tile([128, 1], F32)

            # diag chunks live at low stream positions: load those cols first
            # so compute can start while the rest streams in.
            nc.sync.dma_start(out=RHS[:, :1024], in_=rhs_d[:, :1024])
            nc.sync.dma_start(out=RHS[:, 1024:], in_=rhs_d[:, 1024:])
            nc.sync.dma_start(out=LHS[:], in_=lhs_d[:])
            nc.sync.dma_start(out=TRI[:], in_=tri_d[:])
            nc.sync.dma_start(out=PEN[:], in_=pen_d[:])
            nc.sync.dma_start(out=QCOL[:], in_=qcol_d[:])
            nc.sync.dma_start(out=LIN[:], in_=lin_d[:])

            ia = 0
            id_ = 0
            for (s, c0, g, diag, eng) in SCHEDULE:
                w = g * 512
                PS = psp.tile([128, GROUP_CHUNKS * 512], F32, tag="ps")
                for b in range(g):
                    st, pos = CHUNK_MAP[(s, c0 + b)]
                    v = VMAP[(s, st)]
                    nc.tensor.matmul(PS[:, b * 512:(b + 1) * 512],
                                     LHS[:, v * 128:(v + 1) * 128],
                                     RHS[:, pos * 512:(pos + 1) * 512],
                                     start=True, stop=not (diag and b == 0))
                if diag:
                    nc.tensor.matmul(PS[:, 0:512], TRI[:], PEN[:],
                                     start=False, stop=True)
                if eng == "A":
                    SCR = spool.tile([128, GROUP_CHUNKS * 512], F32, tag="scr")
                    nc.scalar.activation(out=SCR[:, :w], in_=PS[:, :w], func=AF.Relu,
                                         bias=0.0, scale=1.0,
                                         accum_out=ACCA[:, ia:ia + 1])
                    ia += 1
                else:
                    nc.vector.tensor_reduce(out=ACCD[:, id_:id_ + 1], in_=PS[:, :w],
                                            axis=AX.X, op=OP.add,
                                            apply_absolute_value=True)
                    id_ += 1

            # linear terms: accL[:, s] = A_s * q_u + B_s   (A,B per-core inputs)
            for s in range(8):
                nc.vector.tensor_scalar(ACCL[:, s:s + 1], QCOL[:, s:s + 1],
                                        LIN[:, 2 * s:2 * s + 1],
                                        LIN[:, 2 * s + 1:2 * s + 2],
                                        OP.mult, OP.add)

            # combine: out = sum(ACCA) + 0.5*(sum(ACCD) + sum(ACCL))
            nc.vector.tensor_reduce(out=R[:, 0:1], in_=ACCA[:], axis=AX.X, op=OP.add)
            nc.vector.tensor_reduce(out=R[:, 1:2], in_=ACCD[:], axis=AX.X, op=OP.add)
            nc.vector.tensor_reduce(out=R[:, 2:3], in_=ACCL[:], axis=AX.X, op=OP.add)
            nc.vector.tensor_tensor(out=R[:, 1:2], in0=R[:, 1:2], in1=R[:, 2:3],
                                    op=OP.add)
            nc.vector.tensor_scalar(R[:, 1:2], R[:, 1:2], 0.5, None, OP.mult)
            nc.vector.tensor_tensor(out=R[:, 0:1], in0=R[:, 0:1], in1=R[:, 1:2],
                                    op=OP.add)
            nc.vector.tensor_copy(out=OUT[:], in_=R[:, 0:1])
            nc.sync.dma_start(out=out_d[:], in_=OUT[:])

    nc.finalize()
    return nc


def get_program():
    if "nc" not in _CACHE:
        _CACHE["nc"] = build_program()
    return _CACHE["nc"]

# ---------------------------------------------------------------------------
# Host side
# ---------------------------------------------------------------------------

def core_tiles(k):
    """Row-tiles for core k, widest first (matches slot widths)."""
    return sorted([k + 16 * m for m in range(4)] + [15 - k + 16 * m for m in range(4)])


def build_inputs(q):
    """Per-core in_maps for label-sorted preds q (np.float32 [8192])."""
    import ml_dtypes
    BF = ml_dtypes.bfloat16
    q = q.astype(np.float32)
    qb = q.astype(BF)                                     # device-visible q
    rhs1_full = (2.0 - q).astype(np.float32).astype(BF)   # bf16(2 - q_v)
    tri = np.triu(np.ones((128, 128), np.float32)).astype(BF)
    pen = np.zeros((128, 512), np.float32)
    pen[np.arange(128), np.arange(128)] = PENALTY
    pen = pen.astype(BF)

    in_maps = []
    for k in range(8):
        tiles = core_tiles(k)
        rhs = np.zeros((16, STREAM_CAP * 512), BF)
        lhs = np.zeros((16, NVAR * 128), BF)
        qcol = np.zeros((128, 8), np.float32)
        lin = np.zeros((128, 16), np.float32)
        for s, t in enumerate(tiles):
            real = (NBLK - t) * 128                  # real window width in cols
            qcol[:, s] = qb[t * 128:(t + 1) * 128].astype(np.float32)
            # scatter this slot's window into its stream chunks
            for c in range(SLOT_CHUNKS[s]):
                st, pos = CHUNK_MAP[(s, c)]
                lo = c * 512
                take = min(max(real - lo, 0), 512)
                if take > 0:
                    rhs[2 * st, pos * 512: pos * 512 + take] = np.float32(1.0)
                    rhs[2 * st + 1, pos * 512: pos * 512 + take] = \
                        rhs1_full[t * 128 + lo: t * 128 + lo + take]
                # lhsT variant for this (slot, stream)
                v = VMAP[(s, st)]
                lhs[2 * st, v * 128:(v + 1) * 128] = qb[t * 128:(t + 1) * 128]
                lhs[2 * st + 1, v * 128:(v + 1) * 128] = np.float32(1.0)
            # linear terms over this slot's DVE groups (f64 accumulate)
            A = 0.0
            B = 0.0
            for (gs, c0, g, diag, eng) in SCHEDULE:
                if gs != s or eng != "D":
                    continue
                for b in range(g):
                    st, pos = CHUNK_MAP[(s, c0 + b)]
                    A += rhs[2 * st, pos * 512:(pos + 1) * 512].astype(np.float64).sum()
                    B += rhs[2 * st + 1, pos * 512:(pos + 1) * 512].astype(np.float64).sum()
            lin[:, 2 * s] = np.float32(A)
            lin[:, 2 * s + 1] = np.float32(B)
        in_maps.append({"rhs": rhs, "lhs": lhs, "tri": tri, "pen": pen,
                        "qcol": qcol, "linab": lin})
    return in_maps


def tie_correction(labels, q, order):
    """Exact correction for tied labels: reference uses sign()=0 there."""
    ls = labels[order]
    corr = 0.0
    i = 0
    n = len(ls)
    while i < n:
        j = i + 1
        while j < n and ls[j] == ls[i]:
            j += 1
        if j - i > 1:
            for u in range(i, j):
                for v in range(u + 1, j):
                    corr += 2.0 - max(0.0, 2.0 + float(q[u]) - float(q[v]))
        i = j
    return corr


def run(inputs, trace=False):
    from concourse.bass_utils import run_bass_kernel_spmd

    preds = np.asarray(inputs["preds"], dtype=np.float32)
    labels = np.asarray(inputs["labels"], dtype=np.float32)
    order = np.argsort(labels, kind="stable")
    q = preds[order]

    nc = get_program()
    in_maps = build_inputs(q)
    res = run_bass_kernel_spmd(nc, in_maps, core_ids=list(range(8)), trace=trace)
    total = 0.0
    for c in range(8):
        total += res.results[c]["out"].astype(np.float64).sum()
    total += tie_correction(labels, q, order)
    return np.float32(total), res


def kernel(**inputs):
    out, _ = run(inputs, trace=False)
    return out
